# revision 1
# baseline (speedup 1.0000x reference)
"""Bidirectional LSTM (masked time-mean output) as a Trainium2 Bass kernel.

Problem: B=64, T=512, D=300, H=600, Keras-gate-order LSTM both directions,
output = mean over T of [fwd_h ; bwd_h] masked by per-batch lengths.

Strategy (8 NeuronCores, SPMD single program):
  - core c in 0..3: FORWARD direction, batches [16c, 16c+16)
  - core c in 4..7: BACKWARD direction, batches [16(c-4), 16(c-4)+16)
    (backward = same program on host-time-flipped x/mask; the masked
    recurrence c_t = m_t*(f*c+i*g), h_t = m_t*(o*tanh(c)) makes the state
    stay 0 until the sequence becomes active, which reproduces
    tf.reverse_sequence semantics exactly; for forward, steps with m=0
    never contribute to the accumulated sum, so one program serves both.)
  - On device the state is kept transposed (hidden-on-partitions):
    per step zT[2560pad, 16] = U_pad^T(stationary fp16) @ hT(fp16 moving)
    accumulated in PSUM + precomputed xzT (x@W+b, fp32 via float32r
    matmuls at N=512).  Gates/cell in fp32.  hsum accumulates masked h.
  - Output per core: hsum [128, 5*16] -> host divides by T and reassembles.
"""

import sys

if "/opt/trn_rl_repo" not in sys.path:
    sys.path.insert(0, "/opt/trn_rl_repo")

import numpy as np

BATCH = 64
SEQ = 512
DIN = 300
HID = 600

NB = 16          # batches per core
DP = 384         # padded input dim (300 data + row 300 = ones for bias)
HP = 640         # padded hidden per gate
G4P = 4 * HP     # 2560 padded gate columns, gate order (i, f, o, g)
KD = DP // 128   # 3 k-tiles for x@W
KH = HP // 128   # 5 k-tiles for h@U
MT = G4P // 128  # 20 m-tiles
CHUNK = 32       # scan steps per outer-loop chunk
NCHUNK = SEQ // CHUNK

TRACE = False            # test.py sets this for profiled runs
XZ_F32R = True           # use float32r for the x@W matmuls

_CACHE = {}


def _build_nc():
    import concourse.bacc as bacc
    import concourse.bass as bass
    import concourse.tile as tile
    from concourse import mybir

    f32 = mybir.dt.float32
    f32r = mybir.dt.float32r if XZ_F32R else mybir.dt.float32
    f16 = mybir.dt.float16

    nc = bacc.Bacc("TRN2", target_bir_lowering=False, debug=False, num_devices=8)

    xt = nc.dram_tensor("xt", [128, KD, SEQ * NB], f32r, kind="ExternalInput")
    wt = nc.dram_tensor("wt", [128, KD, G4P], f32r, kind="ExternalInput")
    ut = nc.dram_tensor("ut", [128, KH, G4P], f16, kind="ExternalInput")
    mk = nc.dram_tensor("mk", [SEQ, 5 * NB], f32, kind="ExternalInput")
    out = nc.dram_tensor("out", [128, 5 * NB], f32, kind="ExternalOutput")

    SW = CHUNK * NB  # 512 moving columns per xz chunk

    with tile.TileContext(nc) as tc:
        with (
            tc.tile_pool(name="consts", bufs=1) as consts,
            tc.tile_pool(name="state", bufs=1) as state,
            tc.tile_pool(name="xchunk", bufs=2) as xpool,
            tc.tile_pool(name="mchunk", bufs=2) as mpool,
            tc.tile_pool(name="xz", bufs=2) as xzpool,
            tc.tile_pool(name="steps", bufs=3) as spool,
            tc.tile_pool(name="pxz", bufs=2, space="PSUM") as pxz,
            tc.tile_pool(name="pz", bufs=2, space="PSUM") as pz,
        ):
            w_sb = consts.tile([128, KD, G4P], f32r)
            nc.sync.dma_start(out=w_sb, in_=wt[:, :, :])
            u_sb = consts.tile([128, KH, G4P], f16)
            nc.sync.dma_start(out=u_sb, in_=ut[:, :, :])

            hT = state.tile([128, KH * NB], f16)
            c = state.tile([128, KH * NB], f32)
            hsum = state.tile([128, KH * NB], f32)
            nc.vector.memset(hT, 0)
            nc.vector.memset(c, 0)
            nc.vector.memset(hsum, 0)

            with tc.For_i(0, NCHUNK, 1, hint_engines=(mybir.EngineType.PE,)) as ci:
                xchunk = xpool.tile([128, KD, SW], f32r)
                nc.sync.dma_start(
                    out=xchunk, in_=xt[:, :, bass.ds(ci * SW, SW)]
                )
                # per-partition broadcast of the [CHUNK, 5*NB] mask slice
                mchunk = mpool.tile([128, CHUNK, 5 * NB], f32)
                msrc = mk[bass.ds(ci * CHUNK, CHUNK), :]
                nc.gpsimd.dma_start(
                    out=mchunk,
                    in_=bass.AP(
                        tensor=msrc.tensor,
                        offset=msrc.offset,
                        ap=[[0, 128]] + list(msrc.ap),
                    ),
                )

                # ---- input projection for this chunk: xz = (x@W + b)^T ----
                xz = xzpool.tile([128, MT, SW], f32)
                for m in range(MT):
                    px = pxz.tile([128, SW], f32)
                    for k in range(KD):
                        nc.tensor.matmul(
                            px,
                            w_sb[:, k, 128 * m : 128 * (m + 1)],
                            xchunk[:, k, :],
                            start=(k == 0),
                            stop=(k == KD - 1),
                        )
                    nc.scalar.copy(xz[:, m, :], px)

                # ---- serial scan over the chunk ----
                for s in range(CHUNK):
                    zp = pz.tile([128, MT * NB], f32)
                    for m in range(MT):
                        for k in range(KH):
                            nc.tensor.matmul(
                                zp[:, NB * m : NB * (m + 1)],
                                u_sb[:, k, 128 * m : 128 * (m + 1)],
                                hT[:, NB * k : NB * (k + 1)],
                                start=(k == 0),
                                stop=(k == KH - 1),
                            )
                    zs = spool.tile([128, MT * NB], f32)
                    nc.vector.tensor_add(
                        zs.rearrange("p (m b) -> p m b", m=MT),
                        zp.rearrange("p (m b) -> p m b", m=MT),
                        xz[:, :, NB * s : NB * (s + 1)],
                    )
                    # gate columns: i [0,80) f [80,160) o [160,240) g [240,320)
                    nc.scalar.activation(
                        zs[:, 0 : 15 * NB],
                        zs[:, 0 : 15 * NB],
                        mybir.ActivationFunctionType.Sigmoid,
                    )
                    nc.scalar.activation(
                        zs[:, 15 * NB : 20 * NB],
                        zs[:, 15 * NB : 20 * NB],
                        mybir.ActivationFunctionType.Tanh,
                    )
                    ms_ap = mchunk[:, s, :]
                    ig = spool.tile([128, KH * NB], f32)
                    nc.vector.tensor_mul(
                        ig, zs[:, 0 : 5 * NB], zs[:, 15 * NB : 20 * NB]
                    )
                    nc.vector.tensor_mul(c, zs[:, 5 * NB : 10 * NB], c)
                    nc.vector.tensor_add(c, c, ig)
                    nc.vector.tensor_mul(c, c, ms_ap)
                    th = spool.tile([128, KH * NB], f32)
                    nc.scalar.activation(
                        th, c, mybir.ActivationFunctionType.Tanh
                    )
                    nc.vector.tensor_mul(th, th, zs[:, 10 * NB : 15 * NB])
                    nc.vector.tensor_mul(hT, th, ms_ap)
                    nc.vector.tensor_add(hsum, hsum, hT)

            nc.sync.dma_start(out=out[:, :], in_=hsum)

    nc.finalize()
    return nc


def _get_nc():
    if "nc" not in _CACHE:
        _CACHE["nc"] = _build_nc()
    return _CACHE["nc"]


def _pack_x(xs):
    """xs [NB, SEQ, DIN] f32 -> [128, KD, SEQ*NB] with col t*NB+b, ones row."""
    xt = np.zeros((DP, SEQ * NB), np.float32)
    xt[:DIN] = xs.transpose(2, 1, 0).reshape(DIN, SEQ * NB)
    xt[DIN] = 1.0
    return np.ascontiguousarray(
        xt.reshape(KD, 128, SEQ * NB).transpose(1, 0, 2)
    )


_GORDER = [0, 1, 3, 2]  # our gate slots (i,f,o,g) <- reference blocks (i,f,g,o)


def _pack_w(W, b):
    Wp = np.zeros((DP, G4P), np.float32)
    for gi, gr in enumerate(_GORDER):
        Wp[:DIN, HP * gi : HP * gi + HID] = W[:, HID * gr : HID * (gr + 1)]
        Wp[DIN, HP * gi : HP * gi + HID] = b[HID * gr : HID * (gr + 1)]
    return np.ascontiguousarray(Wp.reshape(KD, 128, G4P).transpose(1, 0, 2))


def _pack_u(U):
    Up = np.zeros((HP, G4P), np.float32)
    for gi, gr in enumerate(_GORDER):
        Up[:HID, HP * gi : HP * gi + HID] = U[:, HID * gr : HID * (gr + 1)]
    return np.ascontiguousarray(
        Up.reshape(KH, 128, G4P).transpose(1, 0, 2).astype(np.float16)
    )


def _pack_mask(ms):
    """ms [NB, SEQ] float -> [SEQ, 5*NB] (j-replicated)."""
    return np.ascontiguousarray(np.tile(ms.T.astype(np.float32), (1, KH)))


def kernel(inputs, lengths, training=None, Wf=None, Uf=None, bf=None,
           Wb=None, Ub=None, bb=None, **_unused):
    from concourse.bass_utils import run_bass_kernel_spmd

    x = np.asarray(inputs, np.float32)
    L = np.asarray(lengths).astype(np.int64)
    Wf = np.asarray(Wf, np.float32); Uf = np.asarray(Uf, np.float32)
    bf = np.asarray(bf, np.float32)
    Wb = np.asarray(Wb, np.float32); Ub = np.asarray(Ub, np.float32)
    bb = np.asarray(bb, np.float32)

    mask_full = (np.arange(SEQ)[None, :] < L[:, None]).astype(np.float32)

    wt_f = _pack_w(Wf, bf); ut_f = _pack_u(Uf)
    wt_b = _pack_w(Wb, bb); ut_b = _pack_u(Ub)

    in_maps = []
    for core in range(8):
        fwd = core < 4
        g = core % 4
        xs = x[g * NB : (g + 1) * NB]
        ms = mask_full[g * NB : (g + 1) * NB]
        if not fwd:
            xs = xs[:, ::-1]
            ms = ms[:, ::-1]
        in_maps.append({
            "xt": _pack_x(xs),
            "wt": wt_f if fwd else wt_b,
            "ut": ut_f if fwd else ut_b,
            "mk": _pack_mask(ms),
        })

    nc = _get_nc()
    if TRACE:
        res = run_bass_kernel_spmd(
            nc, in_maps, core_ids=list(range(8)), trace=True
        )
        _CACHE["last_result"] = res
    else:
        res = run_bass_kernel_spmd(nc, in_maps, core_ids=list(range(8)))

    outp = np.zeros((BATCH, 2 * HID), np.float32)
    for core in range(8):
        fwd = core < 4
        g = core % 4
        h = res.results[core]["out"].reshape(128, KH, NB)
        blk = h.transpose(2, 1, 0).reshape(NB, HP)[:, :HID] / float(SEQ)
        col = 0 if fwd else HID
        outp[g * NB : (g + 1) * NB, col : col + HID] = blk
    return outp


# revision 2
# speedup vs baseline: 1.0149x; 1.0149x over previous
"""Bidirectional LSTM (masked time-mean output) as a Trainium2 Bass kernel.

Problem: B=64, T=512, D=300, H=600, Keras-gate-order LSTM both directions,
output = mean over T of [fwd_h ; bwd_h] masked by per-batch lengths.

Strategy (8 NeuronCores, SPMD single program):
  - core c in 0..3: FORWARD direction, batches [16c, 16c+16)
  - core c in 4..7: BACKWARD direction, batches [16(c-4), 16(c-4)+16)
    (backward = same program on host-time-flipped x/mask; the masked
    recurrence c_t = m_t*(f*c+i*g), h_t = m_t*(o*tanh(c)) makes the state
    stay 0 until the sequence becomes active, which reproduces
    tf.reverse_sequence semantics exactly; for forward, steps with m=0
    never contribute to the accumulated sum, so one program serves both.)
  - On device the state is kept transposed (hidden-on-partitions):
    per step zT[2560pad, 16] = U_pad^T(stationary fp16) @ hT(fp16 moving)
    accumulated in PSUM + precomputed xzT (x@W+b, fp32 via float32r
    matmuls at N=512).  Gates/cell in fp32.  hsum accumulates masked h.
  - Output per core: hsum [128, 5*16] -> host divides by T and reassembles.
"""

import sys

if "/opt/trn_rl_repo" not in sys.path:
    sys.path.insert(0, "/opt/trn_rl_repo")

import numpy as np

BATCH = 64
SEQ = 512
DIN = 300
HID = 600

NB = 16          # batches per core
DP = 384         # padded input dim (300 data + row 300 = ones for bias)
HP = 640         # padded hidden per gate
G4P = 4 * HP     # 2560 padded gate columns, gate order (i, f, o, g)
KD = DP // 128   # 3 k-tiles for x@W
KH = HP // 128   # 5 k-tiles for h@U
MT = G4P // 128  # 20 m-tiles
CHUNK = 32       # scan steps per outer-loop chunk
NCHUNK = SEQ // CHUNK

TRACE = False            # test.py sets this for profiled runs
XZ_DT = "f16"            # dtype of the x@W matmul operands: "f32r" | "f32" | "f16"

_CACHE = {}


def _build_nc():
    import concourse.bacc as bacc
    import concourse.bass as bass
    import concourse.tile as tile
    from concourse import mybir

    f32 = mybir.dt.float32
    f32r = {"f32r": mybir.dt.float32r, "f32": mybir.dt.float32,
            "f16": mybir.dt.float16}[XZ_DT]
    f16 = mybir.dt.float16

    nc = bacc.Bacc("TRN2", target_bir_lowering=False, debug=False, num_devices=8)

    xt = nc.dram_tensor("xt", [128, KD, SEQ * NB], f32r, kind="ExternalInput")
    wt = nc.dram_tensor("wt", [128, KD, G4P], f32r, kind="ExternalInput")
    ut = nc.dram_tensor("ut", [128, KH, G4P], f16, kind="ExternalInput")
    mk = nc.dram_tensor("mk", [SEQ, 5 * NB], f32, kind="ExternalInput")
    out = nc.dram_tensor("out", [128, 5 * NB], f32, kind="ExternalOutput")

    SW = CHUNK * NB  # 512 moving columns per xz chunk

    with tile.TileContext(nc) as tc:
        with (
            tc.tile_pool(name="consts", bufs=1) as consts,
            tc.tile_pool(name="state", bufs=1) as state,
            tc.tile_pool(name="xchunk", bufs=2) as xpool,
            tc.tile_pool(name="mchunk", bufs=2) as mpool,
            tc.tile_pool(name="xz", bufs=2) as xzpool,
            tc.tile_pool(name="steps", bufs=3) as spool,
            tc.tile_pool(name="pxz", bufs=2, space="PSUM") as pxz,
            tc.tile_pool(name="pz", bufs=2, space="PSUM") as pz,
        ):
            w_sb = consts.tile([128, KD, G4P], f32r)
            nc.sync.dma_start(out=w_sb, in_=wt[:, :, :])
            u_sb = consts.tile([128, KH, G4P], f16)
            nc.sync.dma_start(out=u_sb, in_=ut[:, :, :])

            hT = state.tile([128, KH * NB], f16)
            c = state.tile([128, KH * NB], f32)
            hsum = state.tile([128, KH * NB], f32)
            nc.vector.memset(hT, 0)
            nc.vector.memset(c, 0)
            nc.vector.memset(hsum, 0)

            with tc.For_i(0, NCHUNK, 1, hint_engines=(mybir.EngineType.PE,)) as ci:
                xchunk = xpool.tile([128, KD, SW], f32r)
                nc.sync.dma_start(
                    out=xchunk, in_=xt[:, :, bass.ds(ci * SW, SW)]
                )
                # per-partition broadcast of the [CHUNK, 5*NB] mask slice
                mchunk = mpool.tile([128, CHUNK, 5 * NB], f32)
                msrc = mk[bass.ds(ci * CHUNK, CHUNK), :]
                nc.gpsimd.dma_start(
                    out=mchunk,
                    in_=bass.AP(
                        tensor=msrc.tensor,
                        offset=msrc.offset,
                        ap=[[0, 128]] + list(msrc.ap),
                    ),
                )

                # ---- input projection for this chunk: xz = (x@W + b)^T ----
                xz = xzpool.tile([128, MT, SW], f32)
                for m in range(MT):
                    px = pxz.tile([128, SW], f32)
                    for k in range(KD):
                        nc.tensor.matmul(
                            px,
                            w_sb[:, k, 128 * m : 128 * (m + 1)],
                            xchunk[:, k, :],
                            start=(k == 0),
                            stop=(k == KD - 1),
                        )
                    nc.scalar.copy(xz[:, m, :], px)

                # ---- serial scan over the chunk ----
                for s in range(CHUNK):
                    zp = pz.tile([128, MT * NB], f32)
                    for m in range(MT):
                        for k in range(KH):
                            nc.tensor.matmul(
                                zp[:, NB * m : NB * (m + 1)],
                                u_sb[:, k, 128 * m : 128 * (m + 1)],
                                hT[:, NB * k : NB * (k + 1)],
                                start=(k == 0),
                                stop=(k == KH - 1),
                            )
                    zs = spool.tile([128, MT * NB], f32)
                    nc.vector.tensor_add(
                        zs.rearrange("p (m b) -> p m b", m=MT),
                        zp.rearrange("p (m b) -> p m b", m=MT),
                        xz[:, :, NB * s : NB * (s + 1)],
                    )
                    # gate columns: i [0,80) f [80,160) o [160,240) g [240,320)
                    nc.scalar.activation(
                        zs[:, 0 : 15 * NB],
                        zs[:, 0 : 15 * NB],
                        mybir.ActivationFunctionType.Sigmoid,
                    )
                    nc.scalar.activation(
                        zs[:, 15 * NB : 20 * NB],
                        zs[:, 15 * NB : 20 * NB],
                        mybir.ActivationFunctionType.Tanh,
                    )
                    ms_ap = mchunk[:, s, :]
                    ig = spool.tile([128, KH * NB], f32)
                    nc.vector.tensor_mul(
                        ig, zs[:, 0 : 5 * NB], zs[:, 15 * NB : 20 * NB]
                    )
                    nc.vector.tensor_mul(c, zs[:, 5 * NB : 10 * NB], c)
                    nc.vector.tensor_add(c, c, ig)
                    nc.vector.tensor_mul(c, c, ms_ap)
                    th = spool.tile([128, KH * NB], f32)
                    nc.scalar.activation(
                        th, c, mybir.ActivationFunctionType.Tanh
                    )
                    nc.vector.tensor_mul(th, th, zs[:, 10 * NB : 15 * NB])
                    nc.vector.tensor_mul(hT, th, ms_ap)
                    nc.vector.tensor_add(hsum, hsum, hT)

            nc.sync.dma_start(out=out[:, :], in_=hsum)

    nc.finalize()
    return nc


def _get_nc():
    if "nc" not in _CACHE:
        _CACHE["nc"] = _build_nc()
    return _CACHE["nc"]


def _pack_x(xs):
    """xs [NB, SEQ, DIN] f32 -> [128, KD, SEQ*NB] with col t*NB+b, ones row."""
    xt = np.zeros((DP, SEQ * NB), np.float32)
    xt[:DIN] = xs.transpose(2, 1, 0).reshape(DIN, SEQ * NB)
    xt[DIN] = 1.0
    return np.ascontiguousarray(
        xt.reshape(KD, 128, SEQ * NB).transpose(1, 0, 2)
    ).astype(_xz_np())


def _xz_np():
    return np.float16 if XZ_DT == "f16" else np.float32


_GORDER = [0, 1, 3, 2]  # our gate slots (i,f,o,g) <- reference blocks (i,f,g,o)


def _pack_w(W, b):
    Wp = np.zeros((DP, G4P), np.float32)
    for gi, gr in enumerate(_GORDER):
        Wp[:DIN, HP * gi : HP * gi + HID] = W[:, HID * gr : HID * (gr + 1)]
        Wp[DIN, HP * gi : HP * gi + HID] = b[HID * gr : HID * (gr + 1)]
    return np.ascontiguousarray(
        Wp.reshape(KD, 128, G4P).transpose(1, 0, 2)
    ).astype(_xz_np())


def _pack_u(U):
    Up = np.zeros((HP, G4P), np.float32)
    for gi, gr in enumerate(_GORDER):
        Up[:HID, HP * gi : HP * gi + HID] = U[:, HID * gr : HID * (gr + 1)]
    return np.ascontiguousarray(
        Up.reshape(KH, 128, G4P).transpose(1, 0, 2).astype(np.float16)
    )


def _pack_mask(ms):
    """ms [NB, SEQ] float -> [SEQ, 5*NB] (j-replicated)."""
    return np.ascontiguousarray(np.tile(ms.T.astype(np.float32), (1, KH)))


def kernel(inputs, lengths, training=None, Wf=None, Uf=None, bf=None,
           Wb=None, Ub=None, bb=None, **_unused):
    from concourse.bass_utils import run_bass_kernel_spmd

    x = np.asarray(inputs, np.float32)
    L = np.asarray(lengths).astype(np.int64)
    Wf = np.asarray(Wf, np.float32); Uf = np.asarray(Uf, np.float32)
    bf = np.asarray(bf, np.float32)
    Wb = np.asarray(Wb, np.float32); Ub = np.asarray(Ub, np.float32)
    bb = np.asarray(bb, np.float32)

    mask_full = (np.arange(SEQ)[None, :] < L[:, None]).astype(np.float32)

    wt_f = _pack_w(Wf, bf); ut_f = _pack_u(Uf)
    wt_b = _pack_w(Wb, bb); ut_b = _pack_u(Ub)

    in_maps = []
    for core in range(8):
        fwd = core < 4
        g = core % 4
        xs = x[g * NB : (g + 1) * NB]
        ms = mask_full[g * NB : (g + 1) * NB]
        if not fwd:
            xs = xs[:, ::-1]
            ms = ms[:, ::-1]
        in_maps.append({
            "xt": _pack_x(xs),
            "wt": wt_f if fwd else wt_b,
            "ut": ut_f if fwd else ut_b,
            "mk": _pack_mask(ms),
        })

    nc = _get_nc()
    if TRACE:
        res = run_bass_kernel_spmd(
            nc, in_maps, core_ids=list(range(8)), trace=True
        )
        _CACHE["last_result"] = res
    else:
        res = run_bass_kernel_spmd(nc, in_maps, core_ids=list(range(8)))

    outp = np.zeros((BATCH, 2 * HID), np.float32)
    for core in range(8):
        fwd = core < 4
        g = core % 4
        h = res.results[core]["out"].reshape(128, KH, NB)
        blk = h.transpose(2, 1, 0).reshape(NB, HP)[:, :HID] / float(SEQ)
        col = 0 if fwd else HID
        outp[g * NB : (g + 1) * NB, col : col + HID] = blk
    return outp


# revision 4
# speedup vs baseline: 1.2050x; 1.1872x over previous
"""Bidirectional LSTM (masked time-mean output) as a Trainium2 Bass kernel.

Problem: B=64, T=512, D=300, H=600, Keras-gate-order LSTM both directions,
output = mean over T of [fwd_h ; bwd_h] masked by per-batch lengths.

Strategy (8 NeuronCores, SPMD single program):
  - core c in 0..3: FORWARD direction, batches [16c, 16c+16)
  - core c in 4..7: BACKWARD direction, batches [16(c-4), 16(c-4)+16)
    (backward = same program on host-time-flipped x/mask; the masked
    recurrence c_t = m_t*(f*c+i*g), h_t = m_t*(o*tanh(c)) makes the state
    stay 0 until the sequence becomes active, which reproduces
    tf.reverse_sequence semantics exactly; for forward, steps with m=0
    never contribute to the accumulated sum, so one program serves both.)
  - On device the state is kept transposed (hidden-on-partitions):
    per step zT[2560pad, 16] = U_pad^T(stationary fp16) @ hT(fp16 moving)
    accumulated in PSUM + precomputed xzT (x@W+b, fp32 via float32r
    matmuls at N=512).  Gates/cell in fp32.  hsum accumulates masked h.
  - Output per core: hsum [128, 5*16] -> host divides by T and reassembles.
"""

import sys

if "/opt/trn_rl_repo" not in sys.path:
    sys.path.insert(0, "/opt/trn_rl_repo")

import numpy as np

BATCH = 64
SEQ = 512
DIN = 300
HID = 600

NB = 16          # batches per core
DP = 384         # padded input dim (300 data + row 300 = ones for bias)
HP = 640         # padded hidden per gate
G4P = 4 * HP     # 2560 padded gate columns, gate order (i, f, o, g)
KD = DP // 128   # 3 k-tiles for x@W
KH = HP // 128   # 5 k-tiles for h@U
MT = G4P // 128  # 20 m-tiles
CHUNK = 32       # scan steps per outer-loop chunk
NCHUNK = SEQ // CHUNK

TRACE = False            # test.py sets this for profiled runs
XZ_DT = "f16"            # dtype of the x@W matmul operands: "f32r" | "f32" | "f16"

_CACHE = {}


def _build_nc():
    import concourse.bacc as bacc
    import concourse.bass as bass
    import concourse.tile as tile
    from concourse import mybir

    f32 = mybir.dt.float32
    f32r = {"f32r": mybir.dt.float32r, "f32": mybir.dt.float32,
            "f16": mybir.dt.float16}[XZ_DT]
    f16 = mybir.dt.float16

    nc = bacc.Bacc("TRN2", target_bir_lowering=False, debug=False, num_devices=8)

    xt = nc.dram_tensor("xt", [128, KD, SEQ * NB], f32r, kind="ExternalInput")
    wt = nc.dram_tensor("wt", [128, KD, G4P], f32r, kind="ExternalInput")
    ut = nc.dram_tensor("ut", [128, KH, G4P], f16, kind="ExternalInput")
    out = nc.dram_tensor("out", [128, 5 * NB], f32, kind="ExternalOutput")

    SW = CHUNK * NB  # 512 moving columns per xz chunk

    with tile.TileContext(nc) as tc:
        with (
            tc.tile_pool(name="consts", bufs=1) as consts,
            tc.tile_pool(name="state", bufs=1) as state,
            tc.tile_pool(name="xchunk", bufs=2) as xpool,
            tc.tile_pool(name="xz", bufs=2) as xzpool,
            tc.tile_pool(name="steps", bufs=3) as spool,
            tc.tile_pool(name="pxz", bufs=2, space="PSUM") as pxz,
            tc.tile_pool(name="pz", bufs=2, space="PSUM") as pz,
        ):
            w_sb = consts.tile([128, KD, G4P], f32r)
            nc.sync.dma_start(out=w_sb, in_=wt[:, :, :])
            u_sb = consts.tile([128, KH, G4P], f16)
            nc.sync.dma_start(out=u_sb, in_=ut[:, :, :])

            hT = state.tile([128, KH * NB], f16)
            c = state.tile([128, KH * NB], f32)
            hsum = state.tile([128, KH * NB], f32)
            nc.vector.memset(hT, 0)
            nc.vector.memset(c, 0)
            nc.vector.memset(hsum, 0)

            with tc.For_i(0, NCHUNK, 1, hint_engines=(mybir.EngineType.PE,)) as ci:
                xchunk = xpool.tile([128, KD, SW], f32r)
                nc.sync.dma_start(
                    out=xchunk, in_=xt[:, :, bass.ds(ci * SW, SW)]
                )
                # ---- input projection for this chunk: xz = (x@W + b)^T ----
                xz = xzpool.tile([128, MT, SW], f32)
                for m in range(MT):
                    px = pxz.tile([128, SW], f32)
                    for k in range(KD):
                        nc.tensor.matmul(
                            px,
                            w_sb[:, k, 128 * m : 128 * (m + 1)],
                            xchunk[:, k, :],
                            start=(k == 0),
                            stop=(k == KD - 1),
                        )
                    nc.scalar.copy(xz[:, m, :], px)

                # ---- serial scan over the chunk ----
                # column-block li = 4*j + g; super-block 0: j in 0..2
                # (li 0..11), super-block 1: j in 3..4 (li 12..19).
                # Gates of SB0 run while the PE does SB1's matmuls.
                for s in range(CHUNK):
                    zps = [
                        pz.tile([128, 12 * NB], f32, name="zp0"),
                        pz.tile([128, 8 * NB], f32, name="zp1"),
                    ]
                    for sb, (j0, jn) in enumerate(((0, 3), (3, 2))):
                        zp = zps[sb]
                        for jj in range(jn):
                            for g in range(4):
                                li = 4 * (j0 + jj) + g
                                lo = 4 * jj + g
                                for k in range(KH):
                                    nc.tensor.matmul(
                                        zp[:, NB * lo : NB * (lo + 1)],
                                        u_sb[:, k, 128 * li : 128 * (li + 1)],
                                        hT[:, NB * k : NB * (k + 1)],
                                        start=(k == 0),
                                        stop=(k == KH - 1),
                                    )
                        # gates for this super-block (hidden blocks j0..j0+jn)
                        zp4 = zp.rearrange("p (j g b) -> p j g b", j=jn, g=4)
                        zs = spool.tile([128, jn * 4 * NB], f32, name=f"zs{sb}")
                        zs4 = zs.rearrange("p (j g b) -> p j g b", j=jn, g=4)
                        nc.vector.tensor_add(
                            zs4,
                            zp4,
                            xz[:, 4 * j0 : 4 * (j0 + jn), NB * s : NB * (s + 1)],
                        )
                        nc.scalar.activation(
                            zs4[:, :, 0:3, :],
                            zs4[:, :, 0:3, :],
                            mybir.ActivationFunctionType.Sigmoid,
                        )
                        nc.scalar.activation(
                            zs4[:, :, 3, :],
                            zs4[:, :, 3, :],
                            mybir.ActivationFunctionType.Tanh,
                        )
                        W = jn * NB
                        cs = c[:, NB * j0 : NB * (j0 + jn)]
                        ms_ap = mchunk[:, s, NB * j0 : NB * (j0 + jn)]
                        ms3 = ms_ap.rearrange("p (j b) -> p j b", j=jn)
                        om = spool.tile([128, W], f32, name=f"om{sb}")
                        nc.vector.tensor_mul(
                            om.rearrange("p (j b) -> p j b", j=jn),
                            zs4[:, :, 2, :],
                            ms3,
                        )
                        ig = spool.tile([128, W], f32, name=f"ig{sb}")
                        nc.vector.tensor_mul(
                            ig.rearrange("p (j b) -> p j b", j=jn),
                            zs4[:, :, 0, :],
                            zs4[:, :, 3, :],
                        )
                        nc.vector.tensor_mul(
                            cs.rearrange("p (j b) -> p j b", j=jn),
                            zs4[:, :, 1, :],
                            cs.rearrange("p (j b) -> p j b", j=jn),
                        )
                        nc.vector.tensor_add(cs, cs, ig)
                        nc.vector.tensor_mul(cs, cs, ms_ap)
                        th = spool.tile([128, W], f32, name=f"th{sb}")
                        nc.scalar.activation(
                            th, cs, mybir.ActivationFunctionType.Tanh
                        )
                        hs = hT[:, NB * j0 : NB * (j0 + jn)]
                        nc.vector.tensor_mul(hs, th, om)
                        nc.vector.tensor_add(
                            hsum[:, NB * j0 : NB * (j0 + jn)],
                            hsum[:, NB * j0 : NB * (j0 + jn)],
                            hs,
                        )

            nc.sync.dma_start(out=out[:, :], in_=hsum)

    nc.finalize()
    return nc


def _get_nc():
    if "nc" not in _CACHE:
        _CACHE["nc"] = _build_nc()
    return _CACHE["nc"]


def _pack_x(xs, ms):
    """xs [NB, SEQ, DIN] f32 -> [128, KD, SEQ*NB] with col t*NB+b.

    Row DIN is the all-ones bias row; row DIN+1 is (1 - mask(t,b)) which,
    against the -MBIG entries in the matching W row, drives the i/f gates to
    sigmoid(-MBIG) ~ 0 at masked steps so cell state and h self-zero."""
    xt = np.zeros((DP, SEQ * NB), np.float32)
    xt[:DIN] = xs.transpose(2, 1, 0).reshape(DIN, SEQ * NB)
    xt[DIN] = 1.0
    xt[DIN + 1] = 1.0 - ms.T.astype(np.float32).reshape(SEQ * NB)
    return np.ascontiguousarray(
        xt.reshape(KD, 128, SEQ * NB).transpose(1, 0, 2)
    ).astype(_xz_np())


def _xz_np():
    return np.float16 if XZ_DT == "f16" else np.float32


_GORDER = [0, 1, 3, 2]  # our gate slots (i,f,o,g) <- reference blocks (i,f,g,o)


def _col_perm():
    """Column-block permutation: position li holds (j, g) with li = 4*j + g
    (j = hidden 128-block 0..4, g = gate 0..3 in (i,f,o,g) order)."""
    perm = []
    for j in range(KH):
        for g in range(4):
            src0 = HP * g + 128 * j
            perm.append((src0, 128))
    return perm


def _permute_cols(A):
    out = np.zeros_like(A)
    for li, (src0, w) in enumerate(_col_perm()):
        out[:, 128 * li : 128 * li + w] = A[:, src0 : src0 + w]
    return out


MBIG = 30.0


def _pack_w(W, b):
    Wp = np.zeros((DP, G4P), np.float32)
    for gi, gr in enumerate(_GORDER):
        Wp[:DIN, HP * gi : HP * gi + HID] = W[:, HID * gr : HID * (gr + 1)]
        Wp[DIN, HP * gi : HP * gi + HID] = b[HID * gr : HID * (gr + 1)]
        if gi in (0, 1):  # i and f gates: mask-kill row
            Wp[DIN + 1, HP * gi : HP * gi + HID] = -MBIG
    Wp = _permute_cols(Wp)
    return np.ascontiguousarray(
        Wp.reshape(KD, 128, G4P).transpose(1, 0, 2)
    ).astype(_xz_np())


def _pack_u(U):
    Up = np.zeros((HP, G4P), np.float32)
    for gi, gr in enumerate(_GORDER):
        Up[:HID, HP * gi : HP * gi + HID] = U[:, HID * gr : HID * (gr + 1)]
    Up = _permute_cols(Up)
    return np.ascontiguousarray(
        Up.reshape(KH, 128, G4P).transpose(1, 0, 2).astype(np.float16)
    )


def kernel(inputs, lengths, training=None, Wf=None, Uf=None, bf=None,
           Wb=None, Ub=None, bb=None, **_unused):
    from concourse.bass_utils import run_bass_kernel_spmd

    x = np.asarray(inputs, np.float32)
    L = np.asarray(lengths).astype(np.int64)
    Wf = np.asarray(Wf, np.float32); Uf = np.asarray(Uf, np.float32)
    bf = np.asarray(bf, np.float32)
    Wb = np.asarray(Wb, np.float32); Ub = np.asarray(Ub, np.float32)
    bb = np.asarray(bb, np.float32)

    mask_full = (np.arange(SEQ)[None, :] < L[:, None]).astype(np.float32)

    wt_f = _pack_w(Wf, bf); ut_f = _pack_u(Uf)
    wt_b = _pack_w(Wb, bb); ut_b = _pack_u(Ub)

    in_maps = []
    for core in range(8):
        fwd = core < 4
        g = core % 4
        xs = x[g * NB : (g + 1) * NB]
        ms = mask_full[g * NB : (g + 1) * NB]
        if not fwd:
            xs = xs[:, ::-1]
            ms = ms[:, ::-1]
        in_maps.append({
            "xt": _pack_x(xs, ms),
            "wt": wt_f if fwd else wt_b,
            "ut": ut_f if fwd else ut_b,
        })

    nc = _get_nc()
    if TRACE:
        res = run_bass_kernel_spmd(
            nc, in_maps, core_ids=list(range(8)), trace=True
        )
        _CACHE["last_result"] = res
    else:
        res = run_bass_kernel_spmd(nc, in_maps, core_ids=list(range(8)))

    outp = np.zeros((BATCH, 2 * HID), np.float32)
    for core in range(8):
        fwd = core < 4
        g = core % 4
        h = res.results[core]["out"].reshape(128, KH, NB)
        blk = h.transpose(2, 1, 0).reshape(NB, HP)[:, :HID] / float(SEQ)
        col = 0 if fwd else HID
        outp[g * NB : (g + 1) * NB, col : col + HID] = blk
    return outp


# revision 5
# speedup vs baseline: 1.2050x; 1.0001x over previous
"""Bidirectional LSTM (masked time-mean output) as a Trainium2 Bass kernel.

Problem: B=64, T=512, D=300, H=600, Keras-gate-order LSTM both directions,
output = mean over T of [fwd_h ; bwd_h] masked by per-batch lengths.

Strategy (8 NeuronCores, SPMD single program):
  - core c in 0..3: FORWARD direction, batches [16c, 16c+16)
  - core c in 4..7: BACKWARD direction, batches [16(c-4), 16(c-4)+16)
    (backward = same program on host-time-flipped x/mask; the masked
    recurrence c_t = m_t*(f*c+i*g), h_t = m_t*(o*tanh(c)) makes the state
    stay 0 until the sequence becomes active, which reproduces
    tf.reverse_sequence semantics exactly; for forward, steps with m=0
    never contribute to the accumulated sum, so one program serves both.)
  - On device the state is kept transposed (hidden-on-partitions):
    per step zT[2560pad, 16] = U_pad^T(stationary fp16) @ hT(fp16 moving)
    accumulated in PSUM + precomputed xzT (x@W+b, fp32 via float32r
    matmuls at N=512).  Gates/cell in fp32.  hsum accumulates masked h.
  - Output per core: hsum [128, 5*16] -> host divides by T and reassembles.
"""

import sys

if "/opt/trn_rl_repo" not in sys.path:
    sys.path.insert(0, "/opt/trn_rl_repo")

import numpy as np

BATCH = 64
SEQ = 512
DIN = 300
HID = 600

NB = 16          # batches per core
DP = 384         # padded input dim (300 data + row 300 = ones for bias)
HP = 640         # padded hidden per gate
G4P = 4 * HP     # 2560 padded gate columns, gate order (i, f, o, g)
KD = DP // 128   # 3 k-tiles for x@W
KH = HP // 128   # 5 k-tiles for h@U
MT = G4P // 128  # 20 m-tiles
CHUNK = 32       # scan steps per outer-loop chunk
NCHUNK = SEQ // CHUNK

TRACE = False            # test.py sets this for profiled runs
XZ_DT = "f16"            # dtype of the x@W matmul operands: "f32r" | "f32" | "f16"

_CACHE = {}


def _build_nc():
    import concourse.bacc as bacc
    import concourse.bass as bass
    import concourse.tile as tile
    from concourse import mybir

    f32 = mybir.dt.float32
    f32r = {"f32r": mybir.dt.float32r, "f32": mybir.dt.float32,
            "f16": mybir.dt.float16}[XZ_DT]
    f16 = mybir.dt.float16

    nc = bacc.Bacc("TRN2", target_bir_lowering=False, debug=False, num_devices=8)

    xt = nc.dram_tensor("xt", [128, KD, SEQ * NB], f32r, kind="ExternalInput")
    wt = nc.dram_tensor("wt", [128, KD, G4P], f32r, kind="ExternalInput")
    ut = nc.dram_tensor("ut", [128, KH, G4P], f16, kind="ExternalInput")
    out = nc.dram_tensor("out", [128, 5 * NB], f32, kind="ExternalOutput")

    SW = CHUNK * NB  # 512 moving columns per xz chunk

    with tile.TileContext(nc) as tc:
        with (
            tc.tile_pool(name="consts", bufs=1) as consts,
            tc.tile_pool(name="state", bufs=1) as state,
            tc.tile_pool(name="xchunk", bufs=2) as xpool,
            tc.tile_pool(name="xz", bufs=2) as xzpool,
            tc.tile_pool(name="steps", bufs=3) as spool,
            tc.tile_pool(name="pxz", bufs=2, space="PSUM") as pxz,
            tc.tile_pool(name="pz", bufs=2, space="PSUM") as pz,
        ):
            w_sb = consts.tile([128, KD, G4P], f32r)
            nc.sync.dma_start(out=w_sb, in_=wt[:, :, :])
            u_sb = consts.tile([128, KH, G4P], f16)
            nc.sync.dma_start(out=u_sb, in_=ut[:, :, :])

            hT = state.tile([128, KH * NB], f16)
            c = state.tile([128, KH * NB], f32)
            hsum = state.tile([128, KH * NB], f32)
            nc.vector.memset(hT, 0)
            nc.vector.memset(c, 0)
            nc.vector.memset(hsum, 0)

            with tc.For_i(0, NCHUNK, 1, hint_engines=(mybir.EngineType.PE,)) as ci:
                xchunk = xpool.tile([128, KD, SW], f32r)
                nc.sync.dma_start(
                    out=xchunk, in_=xt[:, :, bass.ds(ci * SW, SW)]
                )
                # ---- input projection for this chunk: xz = (x@W + b)^T ----
                xz = xzpool.tile([128, MT, SW], f32)
                for m in range(MT):
                    px = pxz.tile([128, SW], f32)
                    for k in range(KD):
                        nc.tensor.matmul(
                            px,
                            w_sb[:, k, 128 * m : 128 * (m + 1)],
                            xchunk[:, k, :],
                            start=(k == 0),
                            stop=(k == KD - 1),
                        )
                    nc.scalar.copy(xz[:, m, :], px)

                # ---- serial scan over the chunk ----
                # column-block li = 4*j + g; super-block 0: j in 0..2
                # (li 0..11), super-block 1: j in 3..4 (li 12..19).
                # Gates of SB0 run while the PE does SB1's matmuls.
                for s in range(CHUNK):
                    zps = [
                        pz.tile([128, 12 * NB], f32, name="zp0"),
                        pz.tile([128, 8 * NB], f32, name="zp1"),
                    ]
                    for sb, (j0, jn) in enumerate(((0, 3), (3, 2))):
                        zp = zps[sb]
                        for jj in range(jn):
                            for g in range(4):
                                li = 4 * (j0 + jj) + g
                                lo = 4 * jj + g
                                for k in range(KH):
                                    nc.tensor.matmul(
                                        zp[:, NB * lo : NB * (lo + 1)],
                                        u_sb[:, k, 128 * li : 128 * (li + 1)],
                                        hT[:, NB * k : NB * (k + 1)],
                                        start=(k == 0),
                                        stop=(k == KH - 1),
                                    )
                        # gates for this super-block (hidden blocks j0..j0+jn)
                        zp4 = zp.rearrange("p (j g b) -> p j g b", j=jn, g=4)
                        zs = spool.tile([128, jn * 4 * NB], f32, name=f"zs{sb}")
                        zs4 = zs.rearrange("p (j g b) -> p j g b", j=jn, g=4)
                        nc.vector.tensor_add(
                            zs4,
                            zp4,
                            xz[:, 4 * j0 : 4 * (j0 + jn), NB * s : NB * (s + 1)],
                        )
                        nc.scalar.activation(
                            zs4[:, :, 0:3, :],
                            zs4[:, :, 0:3, :],
                            mybir.ActivationFunctionType.Sigmoid,
                        )
                        nc.scalar.activation(
                            zs4[:, :, 3, :],
                            zs4[:, :, 3, :],
                            mybir.ActivationFunctionType.Tanh,
                        )
                        W = jn * NB
                        cs = c[:, NB * j0 : NB * (j0 + jn)]
                        ms_ap = mchunk[:, s, NB * j0 : NB * (j0 + jn)]
                        ms3 = ms_ap.rearrange("p (j b) -> p j b", j=jn)
                        om = spool.tile([128, W], f32, name=f"om{sb}")
                        nc.vector.tensor_mul(
                            om.rearrange("p (j b) -> p j b", j=jn),
                            zs4[:, :, 2, :],
                            ms3,
                        )
                        ig = spool.tile([128, W], f32, name=f"ig{sb}")
                        nc.vector.tensor_mul(
                            ig.rearrange("p (j b) -> p j b", j=jn),
                            zs4[:, :, 0, :],
                            zs4[:, :, 3, :],
                        )
                        nc.vector.tensor_mul(
                            cs.rearrange("p (j b) -> p j b", j=jn),
                            zs4[:, :, 1, :],
                            cs.rearrange("p (j b) -> p j b", j=jn),
                        )
                        nc.vector.tensor_add(cs, cs, ig)
                        nc.vector.tensor_mul(cs, cs, ms_ap)
                        th = spool.tile([128, W], f32, name=f"th{sb}")
                        nc.scalar.activation(
                            th, cs, mybir.ActivationFunctionType.Tanh
                        )
                        hs = hT[:, NB * j0 : NB * (j0 + jn)]
                        nc.vector.tensor_mul(hs, th, om)
                        nc.vector.tensor_add(
                            hsum[:, NB * j0 : NB * (j0 + jn)],
                            hsum[:, NB * j0 : NB * (j0 + jn)],
                            hs,
                        )

            nc.sync.dma_start(out=out[:, :], in_=hsum)

    nc.finalize()
    return nc


def _get_nc():
    if "nc" not in _CACHE:
        _CACHE["nc"] = _build_nc()
    return _CACHE["nc"]


def _pack_x(xs, ms):
    """xs [NB, SEQ, DIN] f32 -> [128, KD, SEQ*NB] with col t*NB+b.

    Row DIN is the all-ones bias row; row DIN+1 is (1 - mask(t,b)) which,
    against the -MBIG entries in the matching W row, drives the i/f gates to
    sigmoid(-MBIG) ~ 0 at masked steps so cell state and h self-zero."""
    xt = np.zeros((DP, SEQ * NB), np.float32)
    xt[:DIN] = xs.transpose(2, 1, 0).reshape(DIN, SEQ * NB)
    xt[DIN] = 1.0
    xt[DIN + 1] = 1.0 - ms.T.astype(np.float32).reshape(SEQ * NB)
    return np.ascontiguousarray(
        xt.reshape(KD, 128, SEQ * NB).transpose(1, 0, 2)
    ).astype(_xz_np())


def _xz_np():
    return np.float16 if XZ_DT == "f16" else np.float32


_GORDER = [0, 1, 3, 2]  # our gate slots (i,f,o,g) <- reference blocks (i,f,g,o)


def _col_perm():
    """Column-block order: super-block 0 holds hidden blocks j=0..2 as
    li = 4*0.. with g-major layout (g*3 + j), super-block 1 holds j=3..4
    as li = 12 + g*2 + (j-3).  Gate g in (i,f,o,g) order."""
    perm = []
    for g in range(4):
        for j in range(3):
            perm.append((HP * g + 128 * j, 128))
    for g in range(4):
        for j in range(3, 5):
            perm.append((HP * g + 128 * j, 128))
    return perm


def _permute_cols(A):
    out = np.zeros_like(A)
    for li, (src0, w) in enumerate(_col_perm()):
        out[:, 128 * li : 128 * li + w] = A[:, src0 : src0 + w]
    return out


MBIG = 30.0


def _pack_w(W, b):
    Wp = np.zeros((DP, G4P), np.float32)
    for gi, gr in enumerate(_GORDER):
        Wp[:DIN, HP * gi : HP * gi + HID] = W[:, HID * gr : HID * (gr + 1)]
        Wp[DIN, HP * gi : HP * gi + HID] = b[HID * gr : HID * (gr + 1)]
        if gi in (0, 1):  # i and f gates: mask-kill row
            Wp[DIN + 1, HP * gi : HP * gi + HID] = -MBIG
    Wp = _permute_cols(Wp)
    return np.ascontiguousarray(
        Wp.reshape(KD, 128, G4P).transpose(1, 0, 2)
    ).astype(_xz_np())


def _pack_u(U):
    Up = np.zeros((HP, G4P), np.float32)
    for gi, gr in enumerate(_GORDER):
        Up[:HID, HP * gi : HP * gi + HID] = U[:, HID * gr : HID * (gr + 1)]
    Up = _permute_cols(Up)
    return np.ascontiguousarray(
        Up.reshape(KH, 128, G4P).transpose(1, 0, 2).astype(np.float16)
    )


def kernel(inputs, lengths, training=None, Wf=None, Uf=None, bf=None,
           Wb=None, Ub=None, bb=None, **_unused):
    from concourse.bass_utils import run_bass_kernel_spmd

    x = np.asarray(inputs, np.float32)
    L = np.asarray(lengths).astype(np.int64)
    Wf = np.asarray(Wf, np.float32); Uf = np.asarray(Uf, np.float32)
    bf = np.asarray(bf, np.float32)
    Wb = np.asarray(Wb, np.float32); Ub = np.asarray(Ub, np.float32)
    bb = np.asarray(bb, np.float32)

    mask_full = (np.arange(SEQ)[None, :] < L[:, None]).astype(np.float32)

    wt_f = _pack_w(Wf, bf); ut_f = _pack_u(Uf)
    wt_b = _pack_w(Wb, bb); ut_b = _pack_u(Ub)

    in_maps = []
    for core in range(8):
        fwd = core < 4
        g = core % 4
        xs = x[g * NB : (g + 1) * NB]
        ms = mask_full[g * NB : (g + 1) * NB]
        if not fwd:
            xs = xs[:, ::-1]
            ms = ms[:, ::-1]
        in_maps.append({
            "xt": _pack_x(xs, ms),
            "wt": wt_f if fwd else wt_b,
            "ut": ut_f if fwd else ut_b,
        })

    nc = _get_nc()
    if TRACE:
        res = run_bass_kernel_spmd(
            nc, in_maps, core_ids=list(range(8)), trace=True
        )
        _CACHE["last_result"] = res
    else:
        res = run_bass_kernel_spmd(nc, in_maps, core_ids=list(range(8)))

    outp = np.zeros((BATCH, 2 * HID), np.float32)
    for core in range(8):
        fwd = core < 4
        g = core % 4
        h = res.results[core]["out"].reshape(128, KH, NB)
        blk = h.transpose(2, 1, 0).reshape(NB, HP)[:, :HID] / float(SEQ)
        col = 0 if fwd else HID
        outp[g * NB : (g + 1) * NB, col : col + HID] = blk
    return outp


# revision 6
# speedup vs baseline: 1.2426x; 1.0312x over previous
"""Bidirectional LSTM (masked time-mean output) as a Trainium2 Bass kernel.

Problem: B=64, T=512, D=300, H=600, Keras-gate-order LSTM both directions,
output = mean over T of [fwd_h ; bwd_h] masked by per-batch lengths.

Strategy (8 NeuronCores, SPMD single program):
  - core c in 0..3: FORWARD direction, batches [16c, 16c+16)
  - core c in 4..7: BACKWARD direction, batches [16(c-4), 16(c-4)+16)
    (backward = same program on host-time-flipped x/mask; the masked
    recurrence c_t = m_t*(f*c+i*g), h_t = m_t*(o*tanh(c)) makes the state
    stay 0 until the sequence becomes active, which reproduces
    tf.reverse_sequence semantics exactly; for forward, steps with m=0
    never contribute to the accumulated sum, so one program serves both.)
  - On device the state is kept transposed (hidden-on-partitions):
    per step zT[2560pad, 16] = U_pad^T(stationary fp16) @ hT(fp16 moving)
    accumulated in PSUM + precomputed xzT (x@W+b, fp32 via float32r
    matmuls at N=512).  Gates/cell in fp32.  hsum accumulates masked h.
  - Output per core: hsum [128, 5*16] -> host divides by T and reassembles.
"""

import sys

if "/opt/trn_rl_repo" not in sys.path:
    sys.path.insert(0, "/opt/trn_rl_repo")

import numpy as np

BATCH = 64
SEQ = 512
DIN = 300
HID = 600

NB = 16          # batches per core
DP = 384         # padded input dim (300 data + row 300 = ones for bias)
HP = 640         # padded hidden per gate
G4P = 4 * HP     # 2560 padded gate columns, gate order (i, f, o, g)
KD = DP // 128   # 3 k-tiles for x@W
KH = HP // 128   # 5 k-tiles for h@U
MT = G4P // 128  # 20 m-tiles
CHUNK = 32       # scan steps per outer-loop chunk
NCHUNK = SEQ // CHUNK

TRACE = False            # test.py sets this for profiled runs
XZ_DT = "f16"            # dtype of the x@W matmul operands: "f32r" | "f32" | "f16"

_CACHE = {}


def _build_nc():
    import concourse.bacc as bacc
    import concourse.bass as bass
    import concourse.tile as tile
    from concourse import mybir

    f32 = mybir.dt.float32
    f32r = {"f32r": mybir.dt.float32r, "f32": mybir.dt.float32,
            "f16": mybir.dt.float16}[XZ_DT]
    f16 = mybir.dt.float16

    nc = bacc.Bacc("TRN2", target_bir_lowering=False, debug=False, num_devices=8)

    xt = nc.dram_tensor("xt", [128, KD, SEQ * NB], f32r, kind="ExternalInput")
    wt = nc.dram_tensor("wt", [128, KD, G4P], f32r, kind="ExternalInput")
    ut = nc.dram_tensor("ut", [128, KH, G4P], f16, kind="ExternalInput")
    out = nc.dram_tensor("out", [128, 5 * NB], f32, kind="ExternalOutput")

    SW = CHUNK * NB  # 512 moving columns per xz chunk

    with tile.TileContext(nc) as tc:
        with (
            tc.tile_pool(name="consts", bufs=1) as consts,
            tc.tile_pool(name="state", bufs=1) as state,
            tc.tile_pool(name="xchunk", bufs=2) as xpool,
            tc.tile_pool(name="xz", bufs=2) as xzpool,
            tc.tile_pool(name="steps", bufs=3) as spool,
            tc.tile_pool(name="pxz", bufs=2, space="PSUM") as pxz,
            tc.tile_pool(name="pz", bufs=2, space="PSUM") as pz,
        ):
            w_sb = consts.tile([128, KD, G4P], f32r)
            nc.sync.dma_start(out=w_sb, in_=wt[:, :, :])
            u_sb = consts.tile([128, KH, G4P], f16)
            nc.sync.dma_start(out=u_sb, in_=ut[:, :, :])

            hT = state.tile([128, KH * NB], f16)
            c = state.tile([128, KH * NB], f32)
            hsum = state.tile([128, KH * NB], f32)
            nc.vector.memset(hT, 0)
            nc.vector.memset(c, 0)
            nc.vector.memset(hsum, 0)

            with tc.For_i(0, NCHUNK, 1, hint_engines=(mybir.EngineType.PE,)) as ci:
                xchunk = xpool.tile([128, KD, SW], f32r)
                nc.sync.dma_start(
                    out=xchunk, in_=xt[:, :, bass.ds(ci * SW, SW)]
                )
                # ---- input projection for this chunk: xz = (x@W + b)^T ----
                xz = xzpool.tile([128, MT, SW], f32)
                for m in range(MT):
                    px = pxz.tile([128, SW], f32)
                    for k in range(KD):
                        nc.tensor.matmul(
                            px,
                            w_sb[:, k, 128 * m : 128 * (m + 1)],
                            xchunk[:, k, :],
                            start=(k == 0),
                            stop=(k == KD - 1),
                        )
                    nc.scalar.copy(xz[:, m, :], px)

                # ---- serial scan over the chunk ----
                # column-block li = 4*j + g; super-block 0: j in 0..2
                # (li 0..11), super-block 1: j in 3..4 (li 12..19).
                # Gates of SB0 run while the PE does SB1's matmuls.
                for s in range(CHUNK):
                    zps = [
                        pz.tile([128, 12 * NB], f32, name="zp0"),
                        pz.tile([128, 8 * NB], f32, name="zp1"),
                    ]
                    for sb, (j0, jn) in enumerate(((0, 3), (3, 2))):
                        zp = zps[sb]
                        for jj in range(jn):
                            for g in range(4):
                                li = 4 * (j0 + jj) + g
                                lo = 4 * jj + g
                                for k in range(KH):
                                    nc.tensor.matmul(
                                        zp[:, NB * lo : NB * (lo + 1)],
                                        u_sb[:, k, 128 * li : 128 * (li + 1)],
                                        hT[:, NB * k : NB * (k + 1)],
                                        start=(k == 0),
                                        stop=(k == KH - 1),
                                    )
                        # gates for this super-block (hidden blocks j0..j0+jn)
                        zp4 = zp.rearrange("p (j g b) -> p j g b", j=jn, g=4)
                        zs = spool.tile([128, jn * 4 * NB], f32, name=f"zs{sb}")
                        zs4 = zs.rearrange("p (j g b) -> p j g b", j=jn, g=4)
                        nc.vector.tensor_add(
                            zs4,
                            zp4,
                            xz[:, 4 * j0 : 4 * (j0 + jn), NB * s : NB * (s + 1)],
                        )
                        nc.scalar.activation(
                            zs4[:, :, 0:3, :],
                            zs4[:, :, 0:3, :],
                            mybir.ActivationFunctionType.Sigmoid,
                        )
                        nc.scalar.activation(
                            zs4[:, :, 3, :],
                            zs4[:, :, 3, :],
                            mybir.ActivationFunctionType.Tanh,
                        )
                        W = jn * NB
                        cs = c[:, NB * j0 : NB * (j0 + jn)]
                        ms_ap = mchunk[:, s, NB * j0 : NB * (j0 + jn)]
                        ms3 = ms_ap.rearrange("p (j b) -> p j b", j=jn)
                        om = spool.tile([128, W], f32, name=f"om{sb}")
                        nc.vector.tensor_mul(
                            om.rearrange("p (j b) -> p j b", j=jn),
                            zs4[:, :, 2, :],
                            ms3,
                        )
                        ig = spool.tile([128, W], f32, name=f"ig{sb}")
                        nc.vector.tensor_mul(
                            ig.rearrange("p (j b) -> p j b", j=jn),
                            zs4[:, :, 0, :],
                            zs4[:, :, 3, :],
                        )
                        nc.vector.tensor_mul(
                            cs.rearrange("p (j b) -> p j b", j=jn),
                            zs4[:, :, 1, :],
                            cs.rearrange("p (j b) -> p j b", j=jn),
                        )
                        nc.vector.tensor_add(cs, cs, ig)
                        nc.vector.tensor_mul(cs, cs, ms_ap)
                        th = spool.tile([128, W], f32, name=f"th{sb}")
                        nc.scalar.activation(
                            th, cs, mybir.ActivationFunctionType.Tanh
                        )
                        hs = hT[:, NB * j0 : NB * (j0 + jn)]
                        nc.vector.tensor_mul(hs, th, om)
                        nc.vector.tensor_add(
                            hsum[:, NB * j0 : NB * (j0 + jn)],
                            hsum[:, NB * j0 : NB * (j0 + jn)],
                            hs,
                        )

            nc.sync.dma_start(out=out[:, :], in_=hsum)

    nc.finalize()
    return nc


def _get_nc():
    if "nc" not in _CACHE:
        _CACHE["nc"] = _build_nc()
    return _CACHE["nc"]


def _pack_x(xs, ms):
    """xs [NB, SEQ, DIN] f32 -> [128, KD, SEQ*NB] with col t*NB+b.

    Row DIN is the all-ones bias row; row DIN+1 is (1 - mask(t,b)) which,
    against the -MBIG entries in the matching W row, drives the i/f gates to
    sigmoid(-MBIG) ~ 0 at masked steps so cell state and h self-zero."""
    xt = np.zeros((DP, SEQ * NB), np.float32)
    xt[:DIN] = xs.transpose(2, 1, 0).reshape(DIN, SEQ * NB)
    xt[DIN] = 1.0
    xt[DIN + 1] = 1.0 - ms.T.astype(np.float32).reshape(SEQ * NB)
    return np.ascontiguousarray(
        xt.reshape(KD, 128, SEQ * NB).transpose(1, 0, 2)
    ).astype(_xz_np())


def _xz_np():
    return np.float16 if XZ_DT == "f16" else np.float32


_GORDER = [0, 1, 3, 2]  # our gate slots (i,f,o,g) <- reference blocks (i,f,g,o)


def _col_perm():
    """Column-block order: super-block 0 holds hidden blocks j=0..2 as
    li = 4*0.. with g-major layout (g*3 + j), super-block 1 holds j=3..4
    as li = 12 + g*2 + (j-3).  Gate g in (i,f,o,g) order."""
    perm = []
    for g in range(4):
        for j in range(3):
            perm.append((HP * g + 128 * j, 128))
    for g in range(4):
        for j in range(3, 5):
            perm.append((HP * g + 128 * j, 128))
    return perm


def _permute_cols(A):
    out = np.zeros_like(A)
    for li, (src0, w) in enumerate(_col_perm()):
        out[:, 128 * li : 128 * li + w] = A[:, src0 : src0 + w]
    return out


MBIG = 30.0


def _pack_w(W, b):
    Wp = np.zeros((DP, G4P), np.float32)
    for gi, gr in enumerate(_GORDER):
        sc = 2.0 if gi == 3 else 1.0  # g gate: tanh(x) = 2*sigmoid(2x)-1
        Wp[:DIN, HP * gi : HP * gi + HID] = sc * W[:, HID * gr : HID * (gr + 1)]
        Wp[DIN, HP * gi : HP * gi + HID] = sc * b[HID * gr : HID * (gr + 1)]
        if gi in (0, 1):  # i and f gates: mask-kill row
            Wp[DIN + 1, HP * gi : HP * gi + HID] = -MBIG
    Wp = _permute_cols(Wp)
    return np.ascontiguousarray(
        Wp.reshape(KD, 128, G4P).transpose(1, 0, 2)
    ).astype(_xz_np())


def _pack_u(U):
    Up = np.zeros((HP, G4P), np.float32)
    for gi, gr in enumerate(_GORDER):
        sc = 2.0 if gi == 3 else 1.0
        Up[:HID, HP * gi : HP * gi + HID] = sc * U[:, HID * gr : HID * (gr + 1)]
    Up = _permute_cols(Up)
    return np.ascontiguousarray(
        Up.reshape(KH, 128, G4P).transpose(1, 0, 2).astype(np.float16)
    )


def kernel(inputs, lengths, training=None, Wf=None, Uf=None, bf=None,
           Wb=None, Ub=None, bb=None, **_unused):
    from concourse.bass_utils import run_bass_kernel_spmd

    x = np.asarray(inputs, np.float32)
    L = np.asarray(lengths).astype(np.int64)
    Wf = np.asarray(Wf, np.float32); Uf = np.asarray(Uf, np.float32)
    bf = np.asarray(bf, np.float32)
    Wb = np.asarray(Wb, np.float32); Ub = np.asarray(Ub, np.float32)
    bb = np.asarray(bb, np.float32)

    mask_full = (np.arange(SEQ)[None, :] < L[:, None]).astype(np.float32)

    wt_f = _pack_w(Wf, bf); ut_f = _pack_u(Uf)
    wt_b = _pack_w(Wb, bb); ut_b = _pack_u(Ub)

    in_maps = []
    for core in range(8):
        fwd = core < 4
        g = core % 4
        xs = x[g * NB : (g + 1) * NB]
        ms = mask_full[g * NB : (g + 1) * NB]
        if not fwd:
            xs = xs[:, ::-1]
            ms = ms[:, ::-1]
        in_maps.append({
            "xt": _pack_x(xs, ms),
            "wt": wt_f if fwd else wt_b,
            "ut": ut_f if fwd else ut_b,
        })

    nc = _get_nc()
    if TRACE:
        res = run_bass_kernel_spmd(
            nc, in_maps, core_ids=list(range(8)), trace=True
        )
        _CACHE["last_result"] = res
    else:
        res = run_bass_kernel_spmd(nc, in_maps, core_ids=list(range(8)))

    outp = np.zeros((BATCH, 2 * HID), np.float32)
    for core in range(8):
        fwd = core < 4
        g = core % 4
        h = res.results[core]["out"].reshape(128, KH, NB)
        blk = h.transpose(2, 1, 0).reshape(NB, HP)[:, :HID] / float(SEQ)
        col = 0 if fwd else HID
        outp[g * NB : (g + 1) * NB, col : col + HID] = blk
    return outp
